# revision 37
# baseline (speedup 1.0000x reference)
"""Single-head attention (B=4, T=4096, C=1024, H=64) on 8 trn2 NeuronCores.

Sharding: 8 shards = (batch b, query-half h).  Each core receives x[b]
pre-transposed to xT [C=1024, T=4096]; for h==1 the T columns are rotated by
2048 so that "this core's" 2048 queries are always columns 0:2048 (softmax is
permutation-invariant over keys).  SPMD: identical program on every core.

Design (bf16 matmuls + two-engine exp + paired query chunks):
  All matmul inputs are bf16: 1 cyc/row on the PE (f32r streams at 2 on
  HW) and half the HBM traffic for the x stream.  K^T and V^T come out of
  ONE projection pass ([Wk'|Wv] stationary) into a single merged SBUF
  tile (one eviction copy per block).

  The exp stage (8.4M elements -- ~70us on ScalarE alone) is split
  across ACT and DVE via the Schraudolph bit trick: scores reach PSUM
  pre-scaled by 16*log2(e) (folded into Wk host-side), so
  trunc(s + 16248.5) written as int16 and reinterpreted as bf16 IS
  exp(0.125*s_qk)*(1+-3%).  DVE computes its share of exp with a single
  tensor_scalar_add (f32 PSUM -> int16-bitcast-bf16 SBUF); ACT computes
  exact exp for its share (activation scale un-maps the fold).  The +-3%
  chord error washes out in the softmax average (measured ~9e-3 rel).

  The attention loop walks key tiles with TWO query chunks in flight:
  per key tile, two score matmuls share the K^T stationary and two
  attn@V matmuls share the V stationary (halving LDWEIGHTS pressure);
  the exp ops alternate ACT(chunk0)/DVE(chunk1) so both engines run in
  lockstep at ~the PE's pace, and the attn@V matmuls trail the scores by
  two key tiles so they never stall the PE stream.  The x DMA streams
  c-major in [128,1024] quarters (first quarter split finer) over three
  queues so the first projection block is runnable ASAP.

  The final softmax division happens on the host during the gather: the
  device ships [tc, attn@V^T | exp-sum] ([NTC, 65, 512] per core), so
  the epilogue is one copy + one DMA per query chunk -- no transposes.
"""

import os
import sys

for _p in ("/opt/trn_rl_repo", "/root/.axon_site/_ro/trn_rl_repo"):
    if os.path.isdir(_p) and _p not in sys.path:
        sys.path.append(_p)

import numpy as np
import ml_dtypes

import concourse.bacc as bacc
import concourse.mybir as mybir
import concourse.tile as tile
from concourse.bass_utils import run_bass_kernel_spmd
from concourse.masks import make_identity

B = 4
T = 4096
C = 1024
H = 64
TQ = T // 2  # queries per core
N_CORES = 8

F32 = mybir.dt.float32
BF16 = mybir.dt.bfloat16
I16 = mybir.dt.int16

NC_CH = C // 128  # 8 contraction chunks
NBLK = T // 512  # 8 projection blocks of 512
NPAIR = T // 256  # 16 key pair-blocks of 256
NST = T // 128  # 32 key tiles
NTC = TQ // 512  # 4 query chunks of 512

# exp(0.125*s) ~= bits_as_bf16(trunc(23.083*s + 16248.5))
K_FOLD = 0.125 * 128.0 / np.log(2.0)  # 23.0831
SCH_B = 16248.5  # 2^7 * (127 - 0.0586)
ACT_SCALE = float(np.log(2.0) / 128.0)

EXP = mybir.ActivationFunctionType.Exp
COPY = mybir.ActivationFunctionType.Copy


def _build_module():
    nc = bacc.Bacc("TRN2", target_bir_lowering=False, debug=False, num_devices=N_CORES)

    xT = nc.dram_tensor("xT", [NC_CH, 128, T], BF16, kind="ExternalInput").ap()
    wkv = nc.dram_tensor("wkv", [128, NC_CH, 128], BF16, kind="ExternalInput").ap()
    wq = nc.dram_tensor("wq", [128, NC_CH, H], BF16, kind="ExternalInput").ap()
    # [tc-chunk, attn@V^T | exp-sum row]; transpose + division happen
    # host-side in the gather
    out = nc.dram_tensor("out", [NTC, 65, 512], F32, kind="ExternalOutput").ap()

    with tile.TileContext(nc) as tc:
        with (
            tc.tile_pool(name="const", bufs=1) as const_pool,
            tc.tile_pool(name="xt", bufs=16) as xt_pool,
            tc.tile_pool(name="big", bufs=1) as big_pool,
            tc.tile_pool(name="exp", bufs=8) as exp_pool,
            tc.tile_pool(name="outts", bufs=2) as outts_pool,
            tc.tile_pool(name="p1", bufs=2, space="PSUM") as psum_p1,
            tc.tile_pool(name="psc", bufs=4, space="PSUM") as psum_sc,
            tc.tile_pool(name="pacc", bufs=2, space="PSUM") as psum_acc,
        ):
            # ---- constants ----
            wkv_sb = const_pool.tile([128, NC_CH, 128], BF16, tag="wkv")
            wq_sb = const_pool.tile([128, NC_CH, H], BF16, tag="wq")
            ident_bf = const_pool.tile([128, 128], BF16, tag="ident_bf")
            ident_f32 = const_pool.tile([65, 65], F32, tag="ident_f32")
            scratch = const_pool.tile([128, 1], F32, tag="scratch")
            nc.sync.dma_start(wkv_sb[:], wkv)
            nc.sync.dma_start(wq_sb[:], wq)
            make_identity(nc, ident_bf[:])
            make_identity(nc, ident_f32[:])
            # pull the exp table load off the critical path
            nc.scalar.activation(scratch[:], ident_bf[:, 0:1], EXP)

            # ---- persistent activations ----
            # rows 0:64 = 23.083*K^T, rows 64:128 = V^T (one eviction copy)
            kvt_sb = big_pool.tile([128, T], BF16, tag="kvt")
            qt_sb = big_pool.tile([64, TQ], BF16, tag="qt")
            va = big_pool.tile([128, NST, 65], BF16, tag="va")  # V | ones col
            nc.gpsimd.memset(va[:, :, 64:65], 1.0)

            # ---- x DMA (c-major; quarter 0 split in halves over 4 queues
            # so the first projection block is runnable ASAP) ----
            dma_engines = (nc.sync, nc.gpsimd, nc.scalar)
            all_queues = (nc.sync, nc.gpsimd, nc.scalar)
            xts = {}
            for c in range(NC_CH):
                xt_t = xt_pool.tile([128, 1024], BF16, tag="xt", name=f"xt0_{c}")
                xts[(0, c)] = xt_t
            qi = 0
            for h in range(2):  # quarter 0, 512-col halves, all-c first
                for c in range(NC_CH):
                    all_queues[qi % 3].dma_start(
                        xts[(0, c)][:, h * 512 : (h + 1) * 512],
                        xT[c, :, h * 512 : (h + 1) * 512],
                    )
                    qi += 1
            for quarter in range(1, 4):
                for c in range(NC_CH):
                    xt_t = xt_pool.tile([128, 1024], BF16, tag="xt")
                    all_queues[qi % 3].dma_start(
                        xt_t[:], xT[c, :, quarter * 1024 : (quarter + 1) * 1024]
                    )
                    xts[(quarter, c)] = xt_t
                    qi += 1

            # ---- phase 1: projections per 512-col block ----
            def emit_proj_block(sb):
                quarter, off = divmod(sb * 512, 1024)
                blk = slice(sb * 512, (sb + 1) * 512)
                kv_ps = psum_p1.tile([128, 512], F32, tag="p1")
                for c in range(NC_CH):
                    nc.tensor.matmul(
                        kv_ps[:],
                        wkv_sb[:, c, :],
                        xts[(quarter, c)][:, off : off + 512],
                        start=(c == 0),
                        stop=(c == NC_CH - 1),
                    )
                nc.scalar.activation(kvt_sb[:, blk], kv_ps[:], COPY)
                if sb < NTC:  # queries = keys 0:2048
                    q_ps = psum_p1.tile([64, 512], F32, tag="p1")
                    for c in range(NC_CH):
                        nc.tensor.matmul(
                            q_ps[:],
                            wq_sb[:, c, :],
                            xts[(quarter, c)][:, off : off + 512],
                            start=(c == 0),
                            stop=(c == NC_CH - 1),
                        )
                    nc.vector.tensor_copy(qt_sb[:, blk], q_ps[:])
                # V^T -> V transposes; pairs share one eviction copy
                for j in range(2):
                    st = sb * 4 + 2 * j
                    vt_ps = psum_p1.tile([128, 128], BF16, tag="p1")
                    for k in range(2):
                        nc.tensor.transpose(
                            vt_ps[:, k * 64 : (k + 1) * 64],
                            kvt_sb[64:128, (st + k) * 128 : (st + k + 1) * 128],
                            ident_bf[64:128, 64:128],
                        )
                    nc.vector.tensor_copy(va[:, st : st + 2, 0:64], vt_ps[:])

            # ---- phase 2: attention, two query chunks per pass so each
            # K^T / V stationary load serves two matmuls ----
            acc_tiles = {}
            st_idx = [0]

            def emit_attn2(tp0, tp1, st_lo, st_hi):
                for t in (tp0, tp1):
                    if t not in acc_tiles:
                        acc_tiles[t] = psum_acc.tile(
                            [65, 512], F32, tag="acc", name=f"av{t}"
                        )
                pend = []  # attn@V trails the scores by two key tiles

                def flush(keep=0):
                    while len(pend) > keep:
                        args, kwargs = pend.pop(0)
                        nc.tensor.matmul(*args, **kwargs)

                for st in range(st_lo, st_hi):
                    kslice = kvt_sb[0:64, st * 128 : (st + 1) * 128]
                    scs = {}
                    for t in (tp0, tp1):
                        sc_ps = psum_sc.tile([128, 512], F32, tag="sc")
                        nc.tensor.matmul(
                            sc_ps[:], kslice, qt_sb[:, t * 512 : (t + 1) * 512],
                            start=True, stop=True,
                        )
                        scs[t] = sc_ps
                    flush(keep=4)
                    i = st_idx[0]
                    st_idx[0] += 1
                    exs = {}
                    for ti, t in enumerate((tp0, tp1)):
                        ex = exp_pool.tile([128, 512], BF16, tag="exp")
                        if ti == 0 or i % 11 == 5:
                            nc.scalar.activation(ex[:], scs[t][:], EXP, scale=ACT_SCALE)
                        else:
                            nc.vector.tensor_scalar_add(
                                ex[:].bitcast(I16), scs[t][:], SCH_B
                            )
                        exs[t] = ex
                    for t in (tp0, tp1):
                        pend.append(
                            (
                                (acc_tiles[t][:], va[:, st, :], exs[t][:]),
                                dict(start=(st == 0), stop=(st == NST - 1)),
                            )
                        )
                flush()

            def emit_epilogue(tcp):
                av_ps = acc_tiles[tcp]
                outt_sb = outts_pool.tile([65, 512], F32, tag="outts")
                nc.scalar.activation(outt_sb[:], av_ps[:], COPY)
                for h in range(2):
                    dma_engines[h].dma_start(
                        out[tcp, :, h * 256 : (h + 1) * 256],
                        outt_sb[:, h * 256 : (h + 1) * 256],
                    )

            # emission order: half-0 projections; attention over half-0 keys
            # overlaps the half-1 x DMA + projections.
            for sb in range(4):
                emit_proj_block(sb)
            emit_attn2(0, 1, 0, 16)
            for sb in range(4, NBLK):
                emit_proj_block(sb)
            emit_attn2(0, 1, 16, NST)
            emit_epilogue(0)
            emit_epilogue(1)
            emit_attn2(2, 3, 0, NST)
            emit_epilogue(2)
            emit_epilogue(3)

    nc.compile()
    return nc


_NC_CACHE = None


def _get_module():
    global _NC_CACHE
    if _NC_CACHE is None:
        _NC_CACHE = _build_module()
    return _NC_CACHE


def _make_in_maps(x, Wq, Wk, Wv):
    x64 = np.asarray(x, dtype=np.float64)
    wq64 = np.asarray(Wq, dtype=np.float64)
    wkv64 = np.concatenate(
        [np.asarray(Wk, dtype=np.float64) * K_FOLD, np.asarray(Wv, dtype=np.float64)],
        axis=1,
    )  # [C, 128]
    wkv_t = np.ascontiguousarray(
        wkv64.reshape(NC_CH, 128, 128).transpose(1, 0, 2)
    ).astype(ml_dtypes.bfloat16)
    wq_t = np.ascontiguousarray(
        wq64.reshape(NC_CH, 128, H).transpose(1, 0, 2)
    ).astype(ml_dtypes.bfloat16)
    in_maps = []
    for core in range(N_CORES):
        b, h = divmod(core, 2)
        xt = x64[b].T  # [C, T]
        if h == 1:
            xt = np.concatenate([xt[:, TQ:], xt[:, :TQ]], axis=1)
        xt = np.ascontiguousarray(xt.reshape(NC_CH, 128, T)).astype(ml_dtypes.bfloat16)
        in_maps.append({"xT": xt, "wkv": wkv_t, "wq": wq_t})
    return in_maps


def run(x, Wq, Wk, Wv, **spmd_kwargs):
    """Run on hardware; returns (output, BassKernelResults)."""
    nc = _get_module()
    in_maps = _make_in_maps(x, Wq, Wk, Wv)
    res = run_bass_kernel_spmd(nc, in_maps, core_ids=list(range(N_CORES)), **spmd_kwargs)
    out = np.empty((B, T, H), dtype=np.float32)
    for core in range(N_CORES):
        b, h = divmod(core, 2)
        out[b, h * TQ : (h + 1) * TQ, :] = _postprocess(res.results[core]["out"])
    return out, res


def _postprocess(o):
    """[NTC, 65, 512] device output -> [TQ, H] normalized."""
    o = np.concatenate(list(np.asarray(o, dtype=np.float64)), axis=1)  # [65, TQ]
    return (o[0:H, :] / o[64:65, :]).T.astype(np.float32)


def kernel(x, Wq, Wk, Wv):
    out, _ = run(x, Wq, Wk, Wv)
    return out


# revision 39
# speedup vs baseline: 1.1353x; 1.1353x over previous
"""Single-head attention (B=4, T=4096, C=1024, H=64) on 8 trn2 NeuronCores.

Sharding: 8 shards = (batch b, query-half h).  Each core receives x[b]
pre-transposed to xT [C=1024, T=4096]; for h==1 the T columns are rotated by
2048 so that "this core's" 2048 queries are always columns 0:2048 (softmax is
permutation-invariant over keys).  SPMD: identical program on every core.

Design (bf16 matmuls + two-engine exp + paired query chunks):
  All matmul inputs are bf16: 1 cyc/row on the PE (f32r streams at 2 on
  HW) and half the HBM traffic for the x stream.  K^T and V^T come out of
  ONE projection pass ([Wk'|Wv] stationary) into a single merged SBUF
  tile (one eviction copy per block).

  The exp stage (8.4M elements -- ~70us on ScalarE alone) is split
  across ACT and DVE via the Schraudolph bit trick: scores reach PSUM
  pre-scaled by 16*log2(e) (folded into Wk host-side), so
  trunc(s + 16248.5) written as int16 and reinterpreted as bf16 IS
  exp(0.125*s_qk)*(1+-3%).  DVE computes its share of exp with a single
  tensor_scalar_add (f32 PSUM -> int16-bitcast-bf16 SBUF); ACT computes
  exact exp for its share (activation scale un-maps the fold).  The +-3%
  chord error washes out in the softmax average (measured ~9e-3 rel).

  The attention loop walks key tiles with TWO query chunks in flight:
  per key tile, two score matmuls share the K^T stationary and two
  attn@V matmuls share the V stationary (halving LDWEIGHTS pressure);
  the exp ops alternate ACT(chunk0)/DVE(chunk1) so both engines run in
  lockstep at ~the PE's pace, and the attn@V matmuls trail the scores by
  two key tiles so they never stall the PE stream.  The x DMA streams
  c-major in [128,1024] quarters (first quarter split finer) over three
  queues so the first projection block is runnable ASAP.

  The final softmax division happens on the host during the gather: the
  device ships [tc, attn@V^T | exp-sum] ([NTC, 65, 512] per core), so
  the epilogue is one copy + one DMA per query chunk -- no transposes.
"""

import os
import sys

for _p in ("/opt/trn_rl_repo", "/root/.axon_site/_ro/trn_rl_repo"):
    if os.path.isdir(_p) and _p not in sys.path:
        sys.path.append(_p)

import numpy as np
import ml_dtypes

import concourse.bacc as bacc
import concourse.mybir as mybir
import concourse.tile as tile
from concourse.bass_utils import run_bass_kernel_spmd
from concourse.masks import make_identity

B = 4
T = 4096
C = 1024
H = 64
TQ = T // 2  # queries per core
N_CORES = 8

F32 = mybir.dt.float32
BF16 = mybir.dt.bfloat16
I16 = mybir.dt.int16

NC_CH = C // 128  # 8 contraction chunks
NBLK = T // 512  # 8 projection blocks of 512
NPAIR = T // 256  # 16 key pair-blocks of 256
NST = T // 128  # 32 key tiles
NTC = TQ // 512  # 4 query chunks of 512

# exp(0.125*s) ~= bits_as_bf16(trunc(23.083*s + 16248.5))
K_FOLD = 0.125 * 128.0 / np.log(2.0)  # 23.0831
SCH_B = 16248.5  # 2^7 * (127 - 0.0586)
ACT_SCALE = float(np.log(2.0) / 128.0)

EXP = mybir.ActivationFunctionType.Exp
COPY = mybir.ActivationFunctionType.Copy


def _build_module():
    nc = bacc.Bacc("TRN2", target_bir_lowering=False, debug=False, num_devices=N_CORES)

    xT = nc.dram_tensor("xT", [NC_CH, 128, T], BF16, kind="ExternalInput").ap()
    wkv = nc.dram_tensor("wkv", [128, NC_CH, 128], BF16, kind="ExternalInput").ap()
    wq = nc.dram_tensor("wq", [128, NC_CH, H], BF16, kind="ExternalInput").ap()
    # [tc-chunk, attn@V^T | exp-sum row]; transpose + division happen
    # host-side in the gather
    out = nc.dram_tensor("out", [NTC, 65, 512], F32, kind="ExternalOutput").ap()

    with tile.TileContext(nc) as tc:
        with (
            tc.tile_pool(name="const", bufs=1) as const_pool,
            tc.tile_pool(name="xt", bufs=16) as xt_pool,
            tc.tile_pool(name="big", bufs=1) as big_pool,
            tc.tile_pool(name="exp", bufs=8) as exp_pool,
            tc.tile_pool(name="outts", bufs=2) as outts_pool,
            tc.tile_pool(name="psc", bufs=4, space="PSUM") as psum_sc,
            tc.tile_pool(name="pacc", bufs=4, space="PSUM") as psum_acc,
        ):
            # ---- constants ----
            wkv_sb = const_pool.tile([128, NC_CH, 128], BF16, tag="wkv")
            wq_sb = const_pool.tile([128, NC_CH, H], BF16, tag="wq")
            ident_bf = const_pool.tile([128, 128], BF16, tag="ident_bf")
            ident_f32 = const_pool.tile([65, 65], F32, tag="ident_f32")
            scratch = const_pool.tile([128, 1], F32, tag="scratch")
            nc.sync.dma_start(wkv_sb[:], wkv)
            nc.sync.dma_start(wq_sb[:], wq)
            make_identity(nc, ident_bf[:])
            make_identity(nc, ident_f32[:])
            # pull the exp table load off the critical path
            nc.scalar.activation(scratch[:], ident_bf[:, 0:1], EXP)

            # ---- persistent activations ----
            # rows 0:64 = 23.083*K^T, rows 64:128 = V^T (one eviction copy)
            kvt_sb = big_pool.tile([128, T], BF16, tag="kvt")
            qt_sb = big_pool.tile([64, TQ], BF16, tag="qt")
            va = big_pool.tile([128, NST, 65], BF16, tag="va")  # V | ones col
            nc.gpsimd.memset(va[:, :, 64:65], 1.0)

            # ---- x DMA (c-major; quarter 0 split in halves over 4 queues
            # so the first projection block is runnable ASAP) ----
            dma_engines = (nc.sync, nc.gpsimd, nc.scalar)
            all_queues = (nc.sync, nc.gpsimd, nc.scalar)
            xts = {}
            for c in range(NC_CH):
                xt_t = xt_pool.tile([128, 1024], BF16, tag="xt", name=f"xt0_{c}")
                xts[(0, c)] = xt_t
            qi = 0
            for h in range(2):  # quarter 0, 512-col halves, all-c first
                for c in range(NC_CH):
                    all_queues[qi % 3].dma_start(
                        xts[(0, c)][:, h * 512 : (h + 1) * 512],
                        xT[c, :, h * 512 : (h + 1) * 512],
                    )
                    qi += 1
            for quarter in range(1, 4):
                for c in range(NC_CH):
                    xt_t = xt_pool.tile([128, 1024], BF16, tag="xt")
                    all_queues[qi % 3].dma_start(
                        xt_t[:], xT[c, :, quarter * 1024 : (quarter + 1) * 1024]
                    )
                    xts[(quarter, c)] = xt_t
                    qi += 1

            # ---- PE warmup: dummy matmuls during the x-DMA head keep
            # the HAM clock gate at 2.4GHz for the first real matmuls ----
            warm_ps = psum_acc.tile([128, 128], F32, tag="acc", name="warm")
            for _ in range(36):
                nc.tensor.matmul(
                    warm_ps[:], ident_bf[:], ident_bf[:], start=True, stop=True
                )

            # ---- phase 1: projections per 512-col block ----
            def emit_proj_block(sb):
                quarter, off = divmod(sb * 512, 1024)
                blk = slice(sb * 512, (sb + 1) * 512)
                kv_ps = psum_sc.tile([128, 512], F32, tag="sc")
                for c in range(NC_CH):
                    nc.tensor.matmul(
                        kv_ps[:],
                        wkv_sb[:, c, :],
                        xts[(quarter, c)][:, off : off + 512],
                        start=(c == 0),
                        stop=(c == NC_CH - 1),
                    )
                nc.scalar.activation(kvt_sb[:, blk], kv_ps[:], COPY)
                if sb < NTC:  # queries = keys 0:2048
                    q_ps = psum_sc.tile([64, 512], F32, tag="sc")
                    for c in range(NC_CH):
                        nc.tensor.matmul(
                            q_ps[:],
                            wq_sb[:, c, :],
                            xts[(quarter, c)][:, off : off + 512],
                            start=(c == 0),
                            stop=(c == NC_CH - 1),
                        )
                    nc.vector.tensor_copy(qt_sb[:, blk], q_ps[:])
                # V^T -> V transposes; pairs share one eviction copy
                for j in range(2):
                    st = sb * 4 + 2 * j
                    vt_ps = psum_sc.tile([128, 128], BF16, tag="sc")
                    for k in range(2):
                        nc.tensor.transpose(
                            vt_ps[:, k * 64 : (k + 1) * 64],
                            kvt_sb[64:128, (st + k) * 128 : (st + k + 1) * 128],
                            ident_bf[64:128, 64:128],
                        )
                    nc.vector.tensor_copy(va[:, st : st + 2, 0:64], vt_ps[:])

            # ---- phase 2: attention, four query chunks per pass so each
            # K^T / V stationary load serves four matmuls ----
            acc_tiles = {}
            st_idx = [0]

            def emit_attn4(st_lo, st_hi):
                for t in range(NTC):
                    if t not in acc_tiles:
                        acc_tiles[t] = psum_acc.tile(
                            [65, 512], F32, tag="acc", name=f"av{t}"
                        )
                pend = []  # attn@V trails the scores by two key tiles

                def flush(keep=0):
                    while len(pend) > keep:
                        args, kwargs = pend.pop(0)
                        nc.tensor.matmul(*args, **kwargs)

                for st in range(st_lo, st_hi):
                    kslice = kvt_sb[0:64, st * 128 : (st + 1) * 128]
                    scs = {}
                    for t in range(NTC):
                        sc_ps = psum_sc.tile([128, 512], F32, tag="sc")
                        nc.tensor.matmul(
                            sc_ps[:], kslice, qt_sb[:, t * 512 : (t + 1) * 512],
                            start=True, stop=True,
                        )
                        scs[t] = sc_ps
                    flush(keep=8)
                    i = st_idx[0]
                    st_idx[0] += 1
                    exs = {}
                    for t in range(NTC):
                        ex = exp_pool.tile([128, 512], BF16, tag="exp")
                        if t < 2 or (t == 2 and i % 8 == 3):
                            nc.scalar.activation(ex[:], scs[t][:], EXP, scale=ACT_SCALE)
                        else:
                            nc.vector.tensor_scalar_add(
                                ex[:].bitcast(I16), scs[t][:], SCH_B
                            )
                        exs[t] = ex
                    for t in range(NTC):
                        pend.append(
                            (
                                (acc_tiles[t][:], va[:, st, :], exs[t][:]),
                                dict(start=(st == 0), stop=(st == NST - 1)),
                            )
                        )
                flush()

            def emit_epilogue(tcp):
                av_ps = acc_tiles[tcp]
                outt_sb = outts_pool.tile([65, 512], F32, tag="outts")
                if tcp % 2 == 0:
                    nc.scalar.activation(outt_sb[:], av_ps[:], COPY)
                else:
                    nc.vector.tensor_copy(outt_sb[:], av_ps[:])
                for h in range(2):
                    dma_engines[h].dma_start(
                        out[tcp, :, h * 256 : (h + 1) * 256],
                        outt_sb[:, h * 256 : (h + 1) * 256],
                    )

            # emission order: half-0 projections; attention over half-0 keys
            # overlaps the half-1 x DMA + projections.
            for sb in range(4):
                emit_proj_block(sb)
            emit_attn4(0, 16)
            for sb in range(4, NBLK):
                emit_proj_block(sb)
            emit_attn4(16, NST)
            for tcp in range(NTC):
                emit_epilogue(tcp)

    nc.compile()
    return nc


_NC_CACHE = None


def _get_module():
    global _NC_CACHE
    if _NC_CACHE is None:
        _NC_CACHE = _build_module()
    return _NC_CACHE


def _make_in_maps(x, Wq, Wk, Wv):
    x64 = np.asarray(x, dtype=np.float64)
    wq64 = np.asarray(Wq, dtype=np.float64)
    wkv64 = np.concatenate(
        [np.asarray(Wk, dtype=np.float64) * K_FOLD, np.asarray(Wv, dtype=np.float64)],
        axis=1,
    )  # [C, 128]
    wkv_t = np.ascontiguousarray(
        wkv64.reshape(NC_CH, 128, 128).transpose(1, 0, 2)
    ).astype(ml_dtypes.bfloat16)
    wq_t = np.ascontiguousarray(
        wq64.reshape(NC_CH, 128, H).transpose(1, 0, 2)
    ).astype(ml_dtypes.bfloat16)
    in_maps = []
    for core in range(N_CORES):
        b, h = divmod(core, 2)
        xt = x64[b].T  # [C, T]
        if h == 1:
            xt = np.concatenate([xt[:, TQ:], xt[:, :TQ]], axis=1)
        xt = np.ascontiguousarray(xt.reshape(NC_CH, 128, T)).astype(ml_dtypes.bfloat16)
        in_maps.append({"xT": xt, "wkv": wkv_t, "wq": wq_t})
    return in_maps


def run(x, Wq, Wk, Wv, **spmd_kwargs):
    """Run on hardware; returns (output, BassKernelResults)."""
    nc = _get_module()
    in_maps = _make_in_maps(x, Wq, Wk, Wv)
    res = run_bass_kernel_spmd(nc, in_maps, core_ids=list(range(N_CORES)), **spmd_kwargs)
    out = np.empty((B, T, H), dtype=np.float32)
    for core in range(N_CORES):
        b, h = divmod(core, 2)
        out[b, h * TQ : (h + 1) * TQ, :] = _postprocess(res.results[core]["out"])
    return out, res


def _postprocess(o):
    """[NTC, 65, 512] device output -> [TQ, H] normalized."""
    o = np.concatenate(list(np.asarray(o, dtype=np.float64)), axis=1)  # [65, TQ]
    return (o[0:H, :] / o[64:65, :]).T.astype(np.float32)


def kernel(x, Wq, Wk, Wv):
    out, _ = run(x, Wq, Wk, Wv)
    return out
